# revision 1
# baseline (speedup 1.0000x reference)
"""Trainium2 Bass kernel for nn_Attention_spd (dense transformer attention with
pairwise score bias `spd`, head-drop rescale, and output projection).

Reference computation (b=4, n=1024, dim=512, heads=8, dim_head=64):
    qkv = x @ w_qkv ; q,k,v = split
    dots = q @ k^T * scale + spd
    attn = softmax(dots) * (head_keep * H / sum(head_keep))
    out  = (attn @ v) @ w_out + b_out

Sharding across 8 NeuronCores: core c handles batch c//2 and heads
4*(c%2) .. 4*(c%2)+3 (data parallel on batch x tensor parallel on heads).
Each core computes a partial output projection over its 4 heads; the host
sums the two partials per batch (cheap 2-way reduce) and adds b_out.

Device-side choices:
  - q/k/v/proj matmuls in fp32r (fp32 rounded to 11 mantissa bits, full PE
    speed, ~1e-4 relative error); attention probabilities in bf16.
  - Attention computed transposed: dotsT[j,i] = k @ q^T so the exp'd scores
    are directly the [K=j, N=i] moving operand of attn@v.
  - v augmented with a ones column (M=65): the attn@v matmul also emits the
    softmax denominator (row 64 of the PSUM output).
  - softmax skips max-subtraction (logits ~N(0,2); exp safe in fp32) —
    mathematically identical.
  - exp(dots + spd) = exp(dots) * exp(spd); exp(spd) is precomputed on the
    host in bf16 (halves the dominant DMA stream), and the combine is a bf16
    DVE multiply (2x mode) instead of an f32 add.
  - Head-PAIR batching through 2-bank (128x1024) PSUM tiles: the two heads'
    dots share one PSUM tile so exp / multiply / normalize run as single
    wide ops (ACT is the pacing engine; this halves its per-op overhead).
  - scale folded into wq on host; head_keep rescale folded into w_out rows;
    softmax normalization broadcast via a K=1 fp32r matmul (gpsimd
    partition_broadcast mis-handles base!=0 APs on HW).
  - DMA instruction count minimized (each dma_start costs ~650ns of
    sequencer + shared-HWDGE time), ordered so the first attention phase's
    dependencies land first.
"""
import os
import sys

for _p in ("/opt/trn_rl_repo", os.path.expanduser("~/.axon_site/_ro/trn_rl_repo")):
    if os.path.isdir(_p) and _p not in sys.path:
        sys.path.insert(0, _p)

import numpy as np
import ml_dtypes

import concourse.bass as bass  # noqa: F401
import concourse.tile as tile
from concourse import bacc, mybir
from concourse.bass_utils import run_bass_kernel_spmd

P = 128
B, N, DIM = 4, 1024, 512
HEADS = 8
DIM_HEAD = 64
SCALE = DIM_HEAD ** -0.5
HL = 4          # heads per core (local)
F32 = mybir.dt.float32
F32R = mybir.dt.float32r
BF16 = mybir.dt.bfloat16
ADD = mybir.AluOpType.add
MULT = mybir.AluOpType.mult
EXP = mybir.ActivationFunctionType.Exp

VARIANT = "bf16mul"

_NC = {}


def build_nc(variant=VARIANT):
    """Build the SPMD Bass program (identical on all 8 cores)."""
    nc = bacc.Bacc("TRN2", target_bir_lowering=False, debug=False, num_devices=8)
    xT = nc.dram_tensor("xT", [DIM, N], F32R, kind="ExternalInput").ap()
    # [qm0 | km0 | v | qm1 | km1] so a small early DMA unblocks the first phase
    w3 = nc.dram_tensor("w3", [DIM, 3 * HL * DIM_HEAD], F32R, kind="ExternalInput").ap()
    wo = nc.dram_tensor("wo", [DIM_HEAD, HL, DIM], F32R, kind="ExternalInput").ap()
    # exp(spd) in bf16: [hp, ib, jj, jb, s, ii] — per (hp, ib) contiguous,
    # with the head pair's (s) tiles adjacent so one DVE op covers both
    spdT = nc.dram_tensor("spdT", [2, 2, P, 8, 2, 512], BF16, kind="ExternalInput").ap()
    y = nc.dram_tensor("y", [N, DIM], F32, kind="ExternalOutput").ap()

    from contextlib import ExitStack

    with tile.TileContext(nc) as tc, ExitStack() as ctx:
        const = ctx.enter_context(tc.tile_pool(name="const", bufs=1))
        sb = ctx.enter_context(tc.tile_pool(name="sb", bufs=1))
        spd_pool = ctx.enter_context(tc.tile_pool(name="spd", bufs=3))
        ex_pool = ctx.enter_context(tc.tile_pool(name="ex", bufs=3))
        pr_pool = ctx.enter_context(tc.tile_pool(name="pr", bufs=3))
        nrm_pool = ctx.enter_context(tc.tile_pool(name="nrm", bufs=2))
        ps = ctx.enter_context(tc.tile_pool(name="ps", bufs=2, space="PSUM"))
        ps4 = ctx.enter_context(tc.tile_pool(name="ps4", bufs=4, space="PSUM"))

        # ---- resident loads -------------------------------------------------
        xT_sb = sb.tile([P, 4, N], F32R)
        w3_sb = sb.tile([P, 4, 768], F32R, tag="w3")
        xT_r = xT.rearrange("(kb p) n -> p kb n", p=P)
        w3_r = w3.rearrange("(kb p) m -> p kb m", p=P)
        nc.sync.dma_start(xT_sb[:], xT_r[:])
        nc.sync.dma_start(w3_sb[:, :, 0:256], w3_r[:, :, 0:256])      # q/k m0
        nc.sync.dma_start(w3_sb[:, :, 256:512], w3_r[:, :, 256:512])  # v
        wo_sb = sb.tile([DIM_HEAD, HL, DIM], F32R, tag="wo")

        ones32 = const.tile([P, 1], F32)
        nc.vector.memset(ones32[:], 1.0)
        # ones row at partition 64: lhsT of the K=1 rowsum-reciprocal
        # broadcast matmul (both operands at partition 64 — HW-exact)
        ones65f = const.tile([65, DIM_HEAD], F32, tag="ones65f")
        nc.vector.memset(ones65f[:], 1.0)
        ones65 = const.tile([65, DIM_HEAD], F32R, tag="ones65")
        nc.vector.tensor_copy(ones65[:], ones65f[:])
        wrowf = const.tile([65, 512], F32, tag="wrowf")
        nc.vector.memset(wrowf[:], 1.0)
        wrow = const.tile([65, 512], F32R, tag="wrow")
        nc.vector.tensor_copy(wrow[64:65, :], wrowf[64:65, :])

        # PE warm-up during the initial DMA wait: the PE clock-gate (HAM)
        # starts throttled; ~3.5us of dummy matmuls bring it to full rate
        # before the qkv projections arrive
        warm = ps.tile([P, 1024], F32, tag="big", name="warm")
        for w in range(16):
            nc.tensor.matmul(warm[0:64, 0:512], ones65[64:65, :], wrow[64:65, :],
                             start=True, stop=True)

        # ---- qkv projections ------------------------------------------------
        qT_sb = sb.tile([P, 2, N], F32R, tag="qT")
        kT_sb = sb.tile([P, 2, N], F32R, tag="kT")
        v_aug = sb.tile([P, 8, HL * 65], BF16, tag="vaug")
        v_cols = v_aug[:].rearrange("p jb (h c) -> p jb h c", c=65)
        nc.vector.tensor_copy(
            v_cols[:, :, :, 64:65],
            ones32[:, None, :, None].to_broadcast((P, 8, HL, 1)),
        )

        def qk_proj(qk, dst, m):
            wofs = (512 if m else 0) + qk * 128
            pq = ps.tile([P, 1024], F32, tag="big", name=f"pq_{qk}_{m}")
            for nb in range(2):
                for kb in range(4):
                    nc.tensor.matmul(
                        pq[:, nb * 512:(nb + 1) * 512],
                        w3_sb[:, kb, wofs:wofs + 128],
                        xT_sb[:, kb, nb * 512:(nb + 1) * 512],
                        start=(kb == 0),
                        stop=(kb == 3),
                    )
            nc.scalar.copy(dst[:, m, :], pq[:])

        qk_proj(0, qT_sb, 0)
        qk_proj(1, kT_sb, 0)
        # v: narrow tiles on the 4-slot ring (keeps the wide ring free for
        # the first attention phase's dots)
        for jb in range(8):
            pv = ps4.tile([P, 512], F32, tag="po", name=f"pv_{jb}")
            for kb in range(4):
                nc.tensor.matmul(
                    pv[:, :256],
                    xT_sb[:, kb, jb * 128:(jb + 1) * 128],
                    w3_sb[:, kb, 256:512],
                    start=(kb == 0),
                    stop=(kb == 3),
                )
            nc.vector.tensor_copy(
                v_cols[:, jb, :, :64],
                pv[:, :256].rearrange("p (h c) -> p h c", c=64),
            )

        # first attention phase's spd transfers go on the DMA queue ahead of
        # the late weight loads and the m1 q/k projections
        st00 = spd_pool.tile([P, 8, 2, 512], BF16, tag="spd", name="spd_0_0")
        nc.sync.dma_start(st00[:, 0:4], spdT[0, 0, :, 0:4])
        nc.sync.dma_start(st00[:, 4:8], spdT[0, 0, :, 4:8])
        nc.sync.dma_start(w3_sb[:, :, 512:768], w3_r[:, :, 512:768])  # q/k m1
        nc.sync.dma_start(wo_sb[:], wo[:])

        # ---- attention ------------------------------------------------------
        # scaled attention output, transposed: [d, h, i] (d on partitions)
        scaled = sb.tile([DIM_HEAD, HL, N], F32R, tag="scaled")
        y_all = sb.tile([P, 8, 512], F32, tag="yall")

        def do_norm(po, hp, ib):
            # head-pair normalization: 1/rowsums -> broadcast via K=1
            # matmuls -> rescale into `scaled`
            rc = nrm_pool.tile([65, 1024], F32R, tag="rc", name=f"rc_{hp}_{ib}")
            with nc.allow_low_precision(reason="f32r recip is plenty for softmax denom"):
                for s in range(2):
                    nc.vector.reciprocal(rc[64:65, s * 512:(s + 1) * 512],
                                         po[s][64:65, :])
            pb = ps.tile([P, 1024], F32, tag="big", name=f"pb_{hp}_{ib}")
            for s in range(2):
                nc.tensor.matmul(pb[0:64, s * 512:(s + 1) * 512],
                                 ones65[64:65, :], rc[64:65, s * 512:(s + 1) * 512],
                                 start=True, stop=True)
            bc = nrm_pool.tile([64, 1024], F32, tag="bc", name=f"bc_{hp}_{ib}")
            nc.vector.tensor_copy(bc[:], pb[0:64, :])
            for s in range(2):
                nc.vector.tensor_tensor(
                    scaled[:, 2 * hp + s, ib * 512:(ib + 1) * 512],
                    po[s][0:64, :],
                    bc[:, s * 512:(s + 1) * 512],
                    MULT,
                )

        def proj(iop):
            # narrow tiles from the 4-slot ring (the wide ring keeps feeding
            # the dots/exp stream)
            for half in range(2):
                io = 2 * iop + half
                py = ps4.tile([P, 512], F32, tag="po", name=f"py_{io}")
                for h in range(HL):
                    nc.tensor.matmul(
                        py[:],
                        scaled[:, h, io * 128:(io + 1) * 128],
                        wo_sb[:, h, :],
                        start=(h == 0),
                        stop=(h == HL - 1),
                    )
                nc.vector.tensor_copy(y_all[:, io, :], py[:])
            # gpsimd/SWDGE queue: an output DMA waiting on its copy must not
            # block the spd stream on the SP HWDGE queue
            nc.gpsimd.dma_start(
                y[iop * 256:(iop + 1) * 256, :].rearrange("(half p) q -> p half q", p=P),
                y_all[:, 2 * iop:2 * iop + 2, :])

        prev = None
        for ib in range(2):          # i block of 512 (outer: frees proj early)
            for hp in range(2):      # head pair (local heads 2hp, 2hp+1)
                def m1_chunk(qk, dst):
                    wofs = 512 + qk * 128
                    for nb in range(2):
                        pq1 = ps4.tile([P, 512], F32, tag="po",
                                       name=f"pq1_{qk}_{nb}")
                        for kb in range(4):
                            nc.tensor.matmul(
                                pq1[:],
                                w3_sb[:, kb, wofs:wofs + 128],
                                xT_sb[:, kb, nb * 512:(nb + 1) * 512],
                                start=(kb == 0),
                                stop=(kb == 3),
                            )
                        nc.vector.tensor_copy(
                            dst[:, 1, nb * 512:(nb + 1) * 512], pq1[:])

                if ib == 0 and hp == 0:
                    st = st00
                else:
                    st = spd_pool.tile([P, 8, 2, 512], BF16, tag="spd",
                                       name=f"spd_{hp}_{ib}")
                    nc.sync.dma_start(st[:, 0:4], spdT[hp, ib, :, 0:4])
                    nc.sync.dma_start(st[:, 4:8], spdT[hp, ib, :, 4:8])
                po = [ps4.tile([128, 512], F32, tag="po", name=f"po_{hp}_{ib}_{s}")
                      for s in range(2)]
                for jb in range(8):
                    pd = ps.tile([P, 1024], F32, tag="big", name=f"pd_{hp}_{ib}_{jb}")
                    # the pair's dots back-to-back: disjoint K=64 row groups
                    # can overlap in the PE array
                    for s in range(2):
                        nc.tensor.matmul(
                            pd[:, s * 512:(s + 1) * 512],
                            kT_sb[64 * s:64 * s + 64, hp, jb * 128:(jb + 1) * 128],
                            qT_sb[64 * s:64 * s + 64, hp, ib * 512:(ib + 1) * 512],
                            start=True,
                            stop=True,
                        )
                    # one wide exp + one wide bf16 multiply for both heads
                    ex = ex_pool.tile([P, 1024], BF16, tag="ex", name=f"ex_{hp}_{ib}_{jb}")
                    nc.scalar.activation(ex[:], pd[:], EXP)
                    pr = pr_pool.tile([P, 1024], BF16, tag="pr", name=f"pr_{hp}_{ib}_{jb}")
                    nc.vector.tensor_tensor(
                        pr[:], ex[:],
                        st[:, jb].rearrange("p s i -> p (s i)"),
                        MULT,
                    )
                    for s in range(2):
                        h = 2 * hp + s
                        nc.tensor.matmul(
                            po[s][0:65, :],
                            v_aug[:, jb, h * 65:(h + 1) * 65],
                            pr[:, s * 512:(s + 1) * 512],
                            start=(jb == 0),
                            stop=(jb == 7),
                        )
                    # interleave previous-phase epilogue work into this
                    # phase's mid-stream PE slack instead of its boundary
                    if prev is not None:
                        if jb == 2:
                            do_norm(*prev)
                        if prev[1] == 1:      # prev phase completed its ib
                            if jb == 4:
                                proj(prev[2] * 2)
                            if jb == 6:
                                proj(prev[2] * 2 + 1)
                    if ib == 0 and hp == 0:
                        if jb == 4:
                            m1_chunk(0, qT_sb)
                        if jb == 6:
                            m1_chunk(1, kT_sb)
                prev = (po, hp, ib)

        # flush: last phase's normalization + remaining projections
        p_po, p_hp, p_ib = prev
        do_norm(p_po, p_hp, p_ib)
        proj(2)
        proj(3)

    nc.compile()
    return nc


def _get_nc(variant=VARIANT):
    if variant not in _NC:
        _NC[variant] = build_nc(variant)
    return _NC[variant]


def make_in_maps(x, spd, head_keep, w_qkv, w_out, variant=VARIANT):
    x = np.asarray(x, np.float32)
    spd = np.asarray(spd, np.float32)
    keep = np.asarray(head_keep, np.float32)
    w_qkv = np.asarray(w_qkv, np.float32)
    w_out = np.asarray(w_out, np.float32)
    cfac = keep * (HEADS / keep.sum())

    in_maps = []
    for c in range(8):
        bi, hh = divmod(c, 2)
        h0 = hh * HL
        hs = slice(h0 * DIM_HEAD, (h0 + HL) * DIM_HEAD)
        xT = np.ascontiguousarray(x[bi].T)
        q_cols = w_qkv[:, hs] * np.float32(SCALE)
        k_cols = w_qkv[:, DIM + h0 * DIM_HEAD:DIM + (h0 + HL) * DIM_HEAD]
        v_cols_h = w_qkv[:, 2 * DIM + h0 * DIM_HEAD:2 * DIM + (h0 + HL) * DIM_HEAD]
        w3 = np.ascontiguousarray(np.concatenate(
            [q_cols[:, :128], k_cols[:, :128], v_cols_h,
             q_cols[:, 128:], k_cols[:, 128:]],
            axis=1,
        ))
        wo_rows = w_out[hs, :] * np.repeat(cfac[h0:h0 + HL], DIM_HEAD)[:, None]
        wo = np.ascontiguousarray(wo_rows.reshape(HL, DIM_HEAD, DIM).transpose(1, 0, 2))
        sp = spd[bi, h0:h0 + HL]  # [HL, i, j] with h = 2*hp + s
        # [hp, s, ib, ii, jb, jj] -> [hp, ib, jj, jb, s, ii]
        spdT = sp.reshape(2, 2, 2, 512, 8, 128).transpose(0, 2, 5, 4, 1, 3)
        spdT = np.exp(spdT).astype(ml_dtypes.bfloat16)
        in_maps.append({"xT": xT, "w3": w3, "wo": wo, "spdT": np.ascontiguousarray(spdT)})
    return in_maps


def kernel(x, spd, head_keep, w_qkv, w_out, b_out):
    assert x.shape == (B, N, DIM) and spd.shape == (B, HEADS, N, N)
    nc = _get_nc()
    in_maps = make_in_maps(x, spd, head_keep, w_qkv, w_out)
    res = run_bass_kernel_spmd(nc, in_maps, core_ids=list(range(8)))
    out = np.empty((B, N, DIM), np.float32)
    for bi in range(B):
        out[bi] = res.results[2 * bi]["y"] + res.results[2 * bi + 1]["y"]
    out += np.asarray(b_out, np.float32)[None, None, :]
    return out

